# revision 6
# baseline (speedup 1.0000x reference)
"""LIF neuron (leaky integrate-and-fire) Bass kernel for Trainium2.

Reference semantics (per element, recurrence over time axis T=32):
    mem_t   = tau * mem_{t-1} + x_t
    spike_t = 1.0 if mem_t > vth else 0.0
    mem_t   = mem_t * (1 - spike_t)        # hard reset

Input  x: [16, 32, 65536] f32  ->  Output spikes: [16, 32, 65536] f32.
Sharding: data parallel over batch, 8 cores x 2 batch rows each.

Spikes are binary, so the kernel bit-packs 8 timesteps into one uint8
on-device (exact -- no precision loss): per-core stores shrink from
16 MiB (f32 spikes) to 512 KiB, and the HBM floor drops from
loads+stores (~94us) to loads-only (~44us).

Ring schedule (per pass): all 16x1MiB loads issue first on the sync
HWDGE ring (HBM direction-pure, measured 383 GB/s); the 4x128KiB
packed stores queue behind them by ring FIFO. Full-x SBUF residency
(bufs=NG) means no WAR gating on loads.

Per step t, on a [128, 1024] f32 tile (2 batch x 512 d per partition),
with the membrane chain split into two independent batch-row halves
[128, 512] so the serial acc->mem->acc dependency is latency-hidden:
  DVE  STT: acc_h = (mem_h * tau) + x_h            (per half)
  Pool STT: mem_h' = (acc_h <= vth) * acc_h        (per half)
Spike extraction per PAIR of steps (base-4 Horner, MSB-first):
  ACT  Sign: sgn = sign(acc_even - vth)
  ACT  Relu: u = relu(2*sgn) = 2*s_even            (bf16 out)
  DVE  STT:  v = (acc_odd > vth) + u               (bf16, in {0..3})
  DVE  STT:  packed' = 4*packed + v                (bf16, 4x DVE mode)
After 4 pairs packed in {0..255} holds the window's 8 spikes
MSB-first; Pool converts bf16 -> uint8 and the sync ring stores it.
Host-side np.unpackbits (bitorder='big') recovers exact f32 spikes.
"""

import os
import sys

sys.path.insert(0, "/opt/trn_rl_repo")

import numpy as np

from concourse import bacc, mybir, tile
from concourse.bass_utils import run_bass_kernel_spmd

TAU = 0.2
VTH = 0.5

B, T, D = 16, 32, 65536
N_CORES = 8
B_SH = B // N_CORES          # 2 batch rows per core
P = 128                      # SBUF partitions
FB = D // P                  # 512 d-elems per partition per batch row
F = B_SH * FB                # 1024 free elems per step-tile
NW = T // 8                  # uint8-packed output windows (4)

GS = int(os.environ.get("LIF_GS", "4"))   # timesteps per DMA group
NG = T // GS                 # groups per pass
JG = GS * F                  # per-group free elems (4096)

# Engine knobs (balance tuning): which engine runs mem / v / packed / conv
# NOTE: walrus rejects scalar_tensor_tensor and comparison ops on Pool
# (V3 ISA); Pool legally runs only tensor_tensor add/mult and 1-scalar
# tensor_scalar. So mem/v/packed default to DVE.
E_MEM = os.environ.get("LIF_EMEM", "dve")     # dve only (STT)
E_V = os.environ.get("LIF_EV", "dve")
E_PK = os.environ.get("LIF_EPK", "dve")
E_CV = os.environ.get("LIF_ECV", "dve")
PK_DT = os.environ.get("LIF_PKDT", "bf16")    # bf16 | f32

_progs = {}


def _eng(nc, name):
    return {"dve": nc.vector, "pool": nc.gpsimd, "act": nc.scalar,
            "sync": nc.sync}[name]


def _build_program(hw_loop=None, mode="full"):
    f32 = mybir.dt.float32
    u8 = mybir.dt.uint8
    nc = bacc.Bacc(
        "TRN2",
        target_bir_lowering=False,
        debug=False,
        enable_asserts=False,
        num_devices=N_CORES,
    )
    x = nc.dram_tensor("x", [B_SH, T, D], f32, kind="ExternalInput").ap()
    xr = x.rearrange("b (g tl) (p f) -> g p tl b f", tl=GS, p=P)
    out = nc.dram_tensor("out", [NW, P, F], u8, kind="ExternalOutput").ap()

    with tile.TileContext(nc) as tc:
        with (
            # full-x residency: one buffer per group, reused across passes
            tc.tile_pool(name="xt", bufs=NG) as xp,
            tc.tile_pool(name="acc", bufs=3) as ap_,
            tc.tile_pool(name="sgn", bufs=2) as sgp,
            tc.tile_pool(name="u", bufs=2) as up,
            tc.tile_pool(name="v", bufs=2) as vp,
            tc.tile_pool(name="pk", bufs=4) as pkp,
            tc.tile_pool(name="m0", bufs=2) as mp0,
            tc.tile_pool(name="m1", bufs=2) as mp1,
            tc.tile_pool(name="u8", bufs=2) as u8p,
            tc.tile_pool(name="const", bufs=1) as cp,
        ):
            nvth = cp.tile([P, 1], f32)
            nc.gpsimd.memset(nvth[:], -VTH)

            def body():
                one_pass(nc, xr, out, xp, ap_, sgp, up, vp, pkp,
                         (mp0, mp1), u8p, nvth, mode)

            if hw_loop is None:
                body()
            else:
                # benchmarking only: repeat the full pass in a HW loop so
                # per-pass device time can be fit from wall-clock deltas
                with tc.For_i(0, hw_loop, 1):
                    body()
    nc.compile()
    return nc


def one_pass(nc, xr, out, xp, ap_, sgp, up, vp, pkp, mps, u8p, nvth, mode):
    f32 = mybir.dt.float32
    bf16 = mybir.dt.bfloat16
    u8 = mybir.dt.uint8
    pk_dt = bf16 if PK_DT == "bf16" else f32
    mult = mybir.AluOpType.mult
    add = mybir.AluOpType.add
    is_le = mybir.AluOpType.is_le
    is_gt = mybir.AluOpType.is_gt
    Sign = mybir.ActivationFunctionType.Sign
    Relu = mybir.ActivationFunctionType.Relu

    # Phase 1: issue every load on the sync ring (back-to-back, no WAR)
    xts = []
    for g in range(NG):
        xt = xp.tile([P, JG], f32)
        if mode == "compute":
            nc.gpsimd.memset(xt[:], 0.125)
        else:
            xt_v = xt[:].rearrange("p (tl b f) -> p tl b f", tl=GS, b=B_SH)
            for b in range(B_SH):
                nc.sync.dma_start(out=xt_v[:, :, b], in_=xr[g][:, :, b])
        xts.append(xt)
    if mode == "load":
        return

    # Phase 2: recurrence + packing; stores drain after loads by FIFO
    mem = [None, None]           # per-half membrane tiles [P, FB]
    u = None                     # 2*s_even of current pair (bf16)
    packed = None                # Horner accumulator for current window
    for g in range(NG):
        xt = xts[g]
        for tl in range(GS):
            t = g * GS + tl
            xs = xt[:, tl * F:(tl + 1) * F]
            if t == 0:
                acc = xs         # mem_{-1} = 0 -> acc = x_0
            else:
                acc = ap_.tile([P, F], f32)
                for h in range(B_SH):
                    hs = slice(h * FB, (h + 1) * FB)
                    # acc = (mem * tau) + x_t     (independent per half)
                    nc.vector.scalar_tensor_tensor(
                        out=acc[:, hs], in0=mem[h][:], scalar=TAU,
                        in1=xs[:, hs], op0=mult, op1=add,
                    )
            afull = acc if t == 0 else acc[:]
            if t < T - 1:
                for h in range(B_SH):
                    hs = slice(h * FB, (h + 1) * FB)
                    if E_MEM == "alt":
                        eng = nc.vector if h == 0 else nc.gpsimd
                    else:
                        eng = _eng(nc, E_MEM)
                    m = mps[h].tile([P, FB], f32)
                    # mem' = (acc <= vth) * acc   (hard reset)
                    eng.scalar_tensor_tensor(
                        out=m[:], in0=afull[:, hs], scalar=VTH,
                        in1=afull[:, hs], op0=is_le, op1=mult,
                    )
                    mem[h] = m
            if t % 2 == 0:
                # sgn = sign(acc-vth); u = relu(2*sgn) = 2*s_even
                sgn = sgp.tile([P, F], f32)
                nc.scalar.activation(sgn[:], afull, Sign, bias=nvth[:])
                u = up.tile([P, F], pk_dt)
                nc.scalar.activation(u[:], sgn[:], Relu, scale=2.0)
            else:
                k = (t % 8) // 2     # pair index within window (0..3)
                if k == 0:
                    # first pair writes the fresh window accumulator
                    packed = pkp.tile([P, F], pk_dt)
                    tgt = packed
                else:
                    tgt = vp.tile([P, F], pk_dt)
                # v = (acc_odd > vth) + u     (in {0..3}, exact)
                _eng(nc, E_V).scalar_tensor_tensor(
                    out=tgt[:], in0=afull, scalar=VTH, in1=u[:],
                    op0=is_gt, op1=add,
                )
                if k > 0:
                    # packed' = 4*packed + v   (base-4 Horner, MSB-first)
                    # last pair of the window writes uint8 directly
                    last = t % 8 == 7
                    if last:
                        pnew = u8p.tile([P, F], u8, name="u8w")
                    else:
                        pnew = pkp.tile([P, F], pk_dt, name="pknew")
                    _eng(nc, E_PK).scalar_tensor_tensor(
                        out=pnew[:], in0=packed[:], scalar=4.0,
                        in1=tgt[:], op0=mult, op1=add,
                    )
                    packed = pnew
                    if last and mode == "full":
                        nc.sync.dma_start(out=out[t // 8], in_=pnew[:])


def _get_program(hw_loop=None, mode="full"):
    key = (hw_loop, mode)
    if key not in _progs:
        _progs[key] = _build_program(hw_loop, mode)
    return _progs[key]


# ---- host-side shard/gather ------------------------------------------

def _shard_input(xc):
    return np.ascontiguousarray(xc)


def _gather_output(oc):
    """[NW, P, F] uint8 -> [B_SH, T, D] f32 spikes (exact)."""
    oc = np.asarray(oc, dtype=np.uint8)
    bits = np.unpackbits(oc[..., None], axis=-1)     # [NW,P,F,8] MSB first
    bits = bits.reshape(NW, P, B_SH, FB, 8).transpose(2, 0, 4, 1, 3)
    return np.ascontiguousarray(
        bits.reshape(B_SH, T, D).astype(np.float32)
    )


def device_input(x):
    """Full [B, T, D] -> axis-0 shard-concatenated device input array."""
    return np.ascontiguousarray(np.asarray(x, dtype=np.float32))


def device_output(o):
    """Axis-0 shard-concatenated device output -> full [B, T, D] f32."""
    rows = o.shape[0] // N_CORES
    return np.concatenate(
        [
            _gather_output(o[i * rows:(i + 1) * rows])
            for i in range(N_CORES)
        ],
        axis=0,
    )


def _shard(x):
    return [
        {"x": _shard_input(x[i * B_SH:(i + 1) * B_SH])}
        for i in range(N_CORES)
    ]


def kernel(x):
    x = np.asarray(x, dtype=np.float32)
    assert x.shape == (B, T, D), x.shape
    nc = _get_program()
    res = run_bass_kernel_spmd(nc, _shard(x), list(range(N_CORES)))
    return np.concatenate(
        [_gather_output(res.results[i]["out"]) for i in range(N_CORES)],
        axis=0,
    )


# revision 7
# speedup vs baseline: 1.2713x; 1.2713x over previous
"""LIF neuron (leaky integrate-and-fire) Bass kernel for Trainium2.

Reference semantics (per element, recurrence over time axis T=32):
    mem_t   = tau * mem_{t-1} + x_t
    spike_t = 1.0 if mem_t > vth else 0.0
    mem_t   = mem_t * (1 - spike_t)        # hard reset
Input  x: [16, 32, 65536] f32  ->  Output spikes: [16, 32, 65536] f32.
Sharding: data parallel over batch, 8 cores x 2 batch rows each.

Design (v3, measured-engine-balanced):
  The recurrence needs two 2-tensor DVE ops per step (acc, mem) --
  ~77us/pass on DVE at 1 elem/lane/cycle; that's the compute floor and
  the binding engine.  Spike extraction rides ACT (Sign then Relu,
  ~68us/pass) and writes spikes DIRECTLY as uint8 (exact: {0,1}),
  shrinking stores 4x (16 MiB -> 4 MiB/core).  DMA: 16 MiB loads
  (~52us) + 4 MiB stores (~13us) = ~65us, fully hidden under DVE.
  Pool/PE idle: Pool's Q7 software TT is ~4x slower than DVE and
  walrus rejects STT/comparisons on it; PE fp32 matmul is ~1/4 rate.

  Membrane chain is split into two independent batch-row halves
  [128, 512] so the serial acc->mem->acc dependency pipelines on DVE
  (chain ~46us < 77us throughput).

Per step, on a [128, 1024] step-tile (2 batch x 512 d per partition):
  DVE  STT x2: acc_h = (mem_h * tau) + x_h          (per half, f32)
  DVE  STT x2: mem_h' = (acc_h <= vth) * acc_h      (per half, f32)
  ACT  Sign:   sgn = sign(acc - vth)                (full tile)
  ACT  Relu:   spk_u8 = relu(sgn)                   (uint8 out, exact)
Ring schedule per pass: all 16x1MiB loads issue first on the sync
HWDGE ring; per-group 512KiB uint8 spike stores queue behind them by
ring FIFO (direction-pure).  Full-x SBUF residency (bufs=NG).
"""

import os
import sys

sys.path.insert(0, "/opt/trn_rl_repo")

import numpy as np

from concourse import bacc, mybir, tile
from concourse.bass_utils import run_bass_kernel_spmd

TAU = 0.2
VTH = 0.5

B, T, D = 16, 32, 65536
N_CORES = 8
B_SH = B // N_CORES          # 2 batch rows per core
P = 128                      # SBUF partitions
FB = D // P                  # 512 d-elems per partition per batch row
F = B_SH * FB                # 1024 free elems per step-tile

GS = int(os.environ.get("LIF_GS", "4"))   # timesteps per DMA group
NG = T // GS                 # groups per pass
JG = GS * F                  # per-group free elems (4096)

SPK_DT = os.environ.get("LIF_SPKDT", "u8")   # u8 | f8 | bf16

_progs = {}


def _spk_dt():
    return {"u8": mybir.dt.uint8, "f8": mybir.dt.float8e4,
            "bf16": mybir.dt.bfloat16}[SPK_DT]


def _build_program(hw_loop=None, mode="full"):
    f32 = mybir.dt.float32
    nc = bacc.Bacc(
        "TRN2",
        target_bir_lowering=False,
        debug=False,
        enable_asserts=False,
        num_devices=N_CORES,
    )
    x = nc.dram_tensor("x", [B_SH, T, D], f32, kind="ExternalInput").ap()
    xr = x.rearrange("b (g tl) (p f) -> g p tl b f", tl=GS, p=P)
    out = nc.dram_tensor("out", [NG, P, JG], _spk_dt(),
                         kind="ExternalOutput").ap()

    with tile.TileContext(nc) as tc:
        with (
            # full-x residency: one buffer per group, reused across passes
            tc.tile_pool(name="xt", bufs=NG) as xp,
            tc.tile_pool(name="acc", bufs=3) as ap_,
            tc.tile_pool(name="sgn", bufs=3) as sgp,
            tc.tile_pool(name="spk", bufs=3) as kp,
            tc.tile_pool(name="m0", bufs=2) as mp0,
            tc.tile_pool(name="m1", bufs=2) as mp1,
            tc.tile_pool(name="const", bufs=1) as cp,
        ):
            nvth = cp.tile([P, 1], f32)
            nc.gpsimd.memset(nvth[:], -VTH)

            def body():
                one_pass(nc, xr, out, xp, ap_, sgp, kp, (mp0, mp1),
                         nvth, mode)

            if hw_loop is None:
                body()
            else:
                # benchmarking only: repeat the full pass in a HW loop so
                # per-pass device time can be fit from wall-clock deltas
                with tc.For_i(0, hw_loop, 1):
                    body()
    nc.compile()
    return nc


def one_pass(nc, xr, out, xp, ap_, sgp, kp, mps, nvth, mode):
    f32 = mybir.dt.float32
    mult = mybir.AluOpType.mult
    add = mybir.AluOpType.add
    is_le = mybir.AluOpType.is_le
    Sign = mybir.ActivationFunctionType.Sign
    Relu = mybir.ActivationFunctionType.Relu

    # Phase 1: issue every load on the sync ring (back-to-back, no WAR)
    xts = []
    for g in range(NG):
        xt = xp.tile([P, JG], f32)
        if mode == "compute":
            nc.gpsimd.memset(xt[:], 0.125)
        else:
            xt_v = xt[:].rearrange("p (tl b f) -> p tl b f", tl=GS, b=B_SH)
            for b in range(B_SH):
                nc.sync.dma_start(out=xt_v[:, :, b], in_=xr[g][:, :, b])
        xts.append(xt)
    if mode == "load":
        return

    # Phase 2: recurrence; uint8 spike stores drain after loads by FIFO
    mem = [None, None]           # per-half membrane tiles [P, FB]
    for g in range(NG):
        xt = xts[g]
        spk = kp.tile([P, JG], _spk_dt())
        for tl in range(GS):
            t = g * GS + tl
            xs = xt[:, tl * F:(tl + 1) * F]
            if t == 0:
                acc = xs         # mem_{-1} = 0 -> acc = x_0
            else:
                acc = ap_.tile([P, F], f32)
                for h in range(B_SH):
                    hs = slice(h * FB, (h + 1) * FB)
                    # acc = (mem * tau) + x_t     (independent per half)
                    nc.vector.scalar_tensor_tensor(
                        out=acc[:, hs], in0=mem[h][:], scalar=TAU,
                        in1=xs[:, hs], op0=mult, op1=add,
                    )
            afull = acc if t == 0 else acc[:]
            if t < T - 1:
                for h in range(B_SH):
                    hs = slice(h * FB, (h + 1) * FB)
                    m = mps[h].tile([P, FB], f32)
                    # mem' = (acc <= vth) * acc   (hard reset)
                    nc.vector.scalar_tensor_tensor(
                        out=m[:], in0=afull[:, hs], scalar=VTH,
                        in1=afull[:, hs], op0=is_le, op1=mult,
                    )
                    mem[h] = m
            # sgn = sign(acc-vth); spike = relu(sgn) in {0,1} exactly,
            # written as uint8 directly into the group store tile
            sgn = sgp.tile([P, F], f32)
            nc.scalar.activation(sgn[:], afull, Sign, bias=nvth[:])
            nc.scalar.activation(spk[:, tl * F:(tl + 1) * F], sgn[:], Relu)
        if mode == "full":
            nc.sync.dma_start(out=out[g], in_=spk[:])


def _get_program(hw_loop=None, mode="full"):
    key = (hw_loop, mode)
    if key not in _progs:
        _progs[key] = _build_program(hw_loop, mode)
    return _progs[key]


# ---- host-side shard/gather ------------------------------------------

def _shard_input(xc):
    return np.ascontiguousarray(xc)


def _gather_output(oc):
    """[NG, P, JG] spike-dtype -> [B_SH, T, D] f32 spikes (exact)."""
    oc = np.asarray(oc)
    if oc.dtype == np.uint8 and SPK_DT == "f8":
        oc = (oc != 0)
    elif SPK_DT == "f8":
        oc = (np.asarray(oc).view(np.uint8) != 0)
    sp = oc.reshape(NG, P, GS, B_SH, FB).transpose(3, 0, 2, 1, 4)
    return np.ascontiguousarray(
        sp.reshape(B_SH, T, D).astype(np.float32)
    )


def device_input(x):
    """Full [B, T, D] -> axis-0 shard-concatenated device input array."""
    return np.ascontiguousarray(np.asarray(x, dtype=np.float32))


def device_output(o):
    """Axis-0 shard-concatenated device output -> full [B, T, D] f32."""
    rows = o.shape[0] // N_CORES
    return np.concatenate(
        [
            _gather_output(o[i * rows:(i + 1) * rows])
            for i in range(N_CORES)
        ],
        axis=0,
    )


def _shard(x):
    return [
        {"x": _shard_input(x[i * B_SH:(i + 1) * B_SH])}
        for i in range(N_CORES)
    ]


def kernel(x):
    x = np.asarray(x, dtype=np.float32)
    assert x.shape == (B, T, D), x.shape
    nc = _get_program()
    res = run_bass_kernel_spmd(nc, _shard(x), list(range(N_CORES)))
    return np.concatenate(
        [_gather_output(res.results[i]["out"]) for i in range(N_CORES)],
        axis=0,
    )


# revision 15
# speedup vs baseline: 1.3245x; 1.0418x over previous
"""LIF neuron (leaky integrate-and-fire) Bass kernel for Trainium2.

Reference semantics (per element, recurrence over time axis T=32):
    mem_t   = tau * mem_{t-1} + x_t
    spike_t = 1.0 if mem_t > vth else 0.0
    mem_t   = mem_t * (1 - spike_t)        # hard reset
Input  x: [16, 32, 65536] f32  ->  Output spikes: [16, 32, 65536] f32.
Sharding: data parallel over batch, 8 cores x 2 batch rows each.

Design (v3, measured-engine-balanced):
  The recurrence needs two 2-tensor DVE ops per step (acc, mem) --
  ~77us/pass on DVE at 1 elem/lane/cycle; that's the compute floor and
  the binding engine.  Spike extraction rides ACT: ONE Sign op per
  step writing uint8 directly -- the f32->u8 conversion saturates the
  sign's -1 to 0, so u8(sign(acc-vth)) IS the {0,1} spike exactly
  (~34us ACT).  Stores shrink 4x (16 MiB -> 4 MiB/core) and ride the
  scalar HWDGE ring, overlapping the sync-ring load stream.
  Pool/PE idle: Pool's Q7 software TT is ~4x slower than DVE and
  walrus rejects STT/comparisons on it; PE fp32 matmul is ~1/4 rate.

  Membrane chain is split into two independent batch-row halves
  [128, 512] so the serial acc->mem->acc dependency pipelines on DVE
  (chain ~46us < 77us throughput).

Per step, on a [128, 1024] step-tile (2 batch x 512 d per partition):
  DVE  STT x2: acc_h = (mem_h * tau) + x_h          (per half, f32)
  DVE  STT x2: mem_h' = (acc_h <= vth) * acc_h      (per half, f32)
  ACT  Sign:   spk_u8 = u8(sign(acc - vth))         (uint8 out, exact)
Ring schedule per pass: all 16x1MiB loads issue first on the sync
HWDGE ring; per-group 512KiB uint8 spike stores ride the scalar ring
concurrently.  Full-x SBUF residency (bufs=NG).
"""

import os
import sys

sys.path.insert(0, "/opt/trn_rl_repo")

import numpy as np

from concourse import bacc, mybir, tile
from concourse.bass_utils import run_bass_kernel_spmd

TAU = 0.2
VTH = 0.5

B, T, D = 16, 32, 65536
N_CORES = 8
B_SH = B // N_CORES          # 2 batch rows per core
P = 128                      # SBUF partitions
FB = D // P                  # 512 d-elems per partition per batch row
F = B_SH * FB                # 1024 free elems per step-tile

GS = int(os.environ.get("LIF_GS", "4"))   # timesteps per DMA group
NG = T // GS                 # groups per pass
JG = GS * F                  # per-group free elems (4096)

SPK_DT = os.environ.get("LIF_SPKDT", "u8")   # u8 | f8 | bf16
SPLIT0 = int(os.environ.get("LIF_SPLIT0", "0"))  # per-step DMAs for group 0
ST_RING = os.environ.get("LIF_STRING", "scalar")  # sync | scalar | pool
ACC_FULL = int(os.environ.get("LIF_ACCFULL", "0"))  # acc as one [P,F] op

_progs = {}


def _spk_dt():
    return {"u8": mybir.dt.uint8, "f8": mybir.dt.float8e4,
            "bf16": mybir.dt.bfloat16}[SPK_DT]


def _build_program(hw_loop=None, mode="full"):
    f32 = mybir.dt.float32
    nc = bacc.Bacc(
        "TRN2",
        target_bir_lowering=False,
        debug=False,
        enable_asserts=False,
        num_devices=N_CORES,
    )
    x = nc.dram_tensor("x", [B_SH, T, D], f32, kind="ExternalInput").ap()
    xr = x.rearrange("b (g tl) (p f) -> g p tl b f", tl=GS, p=P)
    out = nc.dram_tensor("out", [NG, P, JG], _spk_dt(),
                         kind="ExternalOutput").ap()

    with tile.TileContext(nc) as tc:
        with (
            # full-x residency: one buffer per group, reused across passes
            tc.tile_pool(name="xt", bufs=NG) as xp,
            tc.tile_pool(name="acc", bufs=3) as ap_,
            tc.tile_pool(name="spk", bufs=3) as kp,
            tc.tile_pool(name="m0", bufs=2) as mp0,
            tc.tile_pool(name="m1", bufs=2) as mp1,
            tc.tile_pool(name="const", bufs=1) as cp,
        ):
            nvth = cp.tile([P, 1], f32)
            nc.gpsimd.memset(nvth[:], -VTH)

            def body():
                one_pass(nc, xr, out, xp, ap_, kp, (mp0, mp1),
                         nvth, mode)

            if hw_loop is None:
                body()
            else:
                # benchmarking only: repeat the full pass in a HW loop so
                # per-pass device time can be fit from wall-clock deltas
                with tc.For_i(0, hw_loop, 1):
                    body()
    nc.compile()
    return nc


def one_pass(nc, xr, out, xp, ap_, kp, mps, nvth, mode):
    f32 = mybir.dt.float32
    mult = mybir.AluOpType.mult
    add = mybir.AluOpType.add
    is_le = mybir.AluOpType.is_le
    Sign = mybir.ActivationFunctionType.Sign

    # Phase 1: issue every load on the sync ring (back-to-back, no WAR).
    # Group 0 loads are split per-step so the first acc can start after
    # a 256 KiB DMA instead of a 1 MiB one (ramp cut).
    xts = []
    for g in range(NG):
        xt = xp.tile([P, JG], f32)
        if mode == "compute":
            nc.gpsimd.memset(xt[:], 0.125)
        else:
            xt_v = xt[:].rearrange("p (tl b f) -> p tl b f", tl=GS, b=B_SH)
            if g == 0 and SPLIT0:
                for tl in range(GS):
                    for b in range(B_SH):
                        nc.sync.dma_start(out=xt_v[:, tl, b],
                                          in_=xr[g][:, tl, b])
            else:
                for b in range(B_SH):
                    nc.sync.dma_start(out=xt_v[:, :, b], in_=xr[g][:, :, b])
        xts.append(xt)
    if mode == "load":
        return

    # Phase 2: recurrence; uint8 spike stores drain after loads by FIFO
    mem = [None, None]           # per-half membrane tiles [P, FB]
    for g in range(NG):
        xt = xts[g]
        spk = kp.tile([P, JG], _spk_dt())
        for tl in range(GS):
            t = g * GS + tl
            xs = xt[:, tl * F:(tl + 1) * F]
            if t == 0:
                acc = xs         # mem_{-1} = 0 -> acc = x_0
            else:
                acc = ap_.tile([P, F], f32)
                if ACC_FULL:
                    # one full-tile op; mem is a full tile w/ half writes
                    nc.vector.scalar_tensor_tensor(
                        out=acc[:], in0=mem[0][:], scalar=TAU,
                        in1=xs, op0=mult, op1=add,
                    )
                else:
                    for h in range(B_SH):
                        hs = slice(h * FB, (h + 1) * FB)
                        # acc = (mem * tau) + x_t  (independent per half)
                        nc.vector.scalar_tensor_tensor(
                            out=acc[:, hs], in0=mem[h][:, hs] if ACC_FULL
                            else mem[h][:], scalar=TAU,
                            in1=xs[:, hs], op0=mult, op1=add,
                        )
            afull = acc if t == 0 else acc[:]
            if t < T - 1:
                if ACC_FULL:
                    m = mps[0].tile([P, F], f32, name="memf")
                    for h in range(B_SH):
                        hs = slice(h * FB, (h + 1) * FB)
                        # mem' = (acc <= vth) * acc   (hard reset)
                        nc.vector.scalar_tensor_tensor(
                            out=m[:, hs], in0=afull[:, hs], scalar=VTH,
                            in1=afull[:, hs], op0=is_le, op1=mult,
                        )
                    mem = [m, m]
                else:
                    for h in range(B_SH):
                        hs = slice(h * FB, (h + 1) * FB)
                        m = mps[h].tile([P, FB], f32)
                        # mem' = (acc <= vth) * acc   (hard reset)
                        nc.vector.scalar_tensor_tensor(
                            out=m[:], in0=afull[:, hs], scalar=VTH,
                            in1=afull[:, hs], op0=is_le, op1=mult,
                        )
                        mem[h] = m
            # spike = sign(acc-vth) written directly as uint8: the f32->u8
            # conversion saturates -1 to 0, so u8(sgn) is exactly the
            # {0,1} spike (verified incl. threshold-boundary values)
            nc.scalar.activation(spk[:, tl * F:(tl + 1) * F], afull,
                                 Sign, bias=nvth[:])
        if mode == "full":
            st_eng = {"sync": nc.sync, "scalar": nc.scalar,
                      "pool": nc.gpsimd}[ST_RING]
            st_eng.dma_start(out=out[g], in_=spk[:])


def _get_program(hw_loop=None, mode="full"):
    key = (hw_loop, mode)
    if key not in _progs:
        _progs[key] = _build_program(hw_loop, mode)
    return _progs[key]


# ---- host-side shard/gather ------------------------------------------

def _shard_input(xc):
    return np.ascontiguousarray(xc)


def _gather_output(oc):
    """[NG, P, JG] spike-dtype -> [B_SH, T, D] f32 spikes (exact)."""
    oc = np.asarray(oc)
    if oc.dtype == np.uint8 and SPK_DT == "f8":
        oc = (oc != 0)
    elif SPK_DT == "f8":
        oc = (np.asarray(oc).view(np.uint8) != 0)
    sp = oc.reshape(NG, P, GS, B_SH, FB).transpose(3, 0, 2, 1, 4)
    return np.ascontiguousarray(
        sp.reshape(B_SH, T, D).astype(np.float32)
    )


def device_input(x):
    """Full [B, T, D] -> axis-0 shard-concatenated device input array."""
    return np.ascontiguousarray(np.asarray(x, dtype=np.float32))


def device_output(o):
    """Axis-0 shard-concatenated device output -> full [B, T, D] f32."""
    rows = o.shape[0] // N_CORES
    return np.concatenate(
        [
            _gather_output(o[i * rows:(i + 1) * rows])
            for i in range(N_CORES)
        ],
        axis=0,
    )


def _shard(x):
    return [
        {"x": _shard_input(x[i * B_SH:(i + 1) * B_SH])}
        for i in range(N_CORES)
    ]


def kernel(x):
    x = np.asarray(x, dtype=np.float32)
    assert x.shape == (B, T, D), x.shape
    nc = _get_program()
    res = run_bass_kernel_spmd(nc, _shard(x), list(range(N_CORES)))
    return np.concatenate(
        [_gather_output(res.results[i]["out"]) for i in range(N_CORES)],
        axis=0,
    )


# revision 18
# speedup vs baseline: 1.3278x; 1.0025x over previous
"""LIF neuron (leaky integrate-and-fire) Bass kernel for Trainium2.

Reference semantics (per element, recurrence over time axis T=32):
    mem_t   = tau * mem_{t-1} + x_t
    spike_t = 1.0 if mem_t > vth else 0.0
    mem_t   = mem_t * (1 - spike_t)        # hard reset
Input  x: [16, 32, 65536] f32  ->  Output spikes: [16, 32, 65536] f32.
Sharding: data parallel over batch, 8 cores x 2 batch rows each.

Design (v3, measured-engine-balanced):
  The recurrence needs two 2-tensor DVE ops per step (acc, mem) --
  ~77us/pass on DVE at 1 elem/lane/cycle; that's the compute floor and
  the binding engine.  Spike extraction rides ACT: ONE Sign op per
  step writing uint8 directly -- the f32->u8 conversion saturates the
  sign's -1 to 0, so u8(sign(acc-vth)) IS the {0,1} spike exactly
  (~34us ACT).  Stores shrink 4x (16 MiB -> 4 MiB/core) and ride the
  scalar HWDGE ring, overlapping the sync-ring load stream.
  Pool/PE idle: Pool's Q7 software TT is ~4x slower than DVE and
  walrus rejects STT/comparisons on it; PE fp32 matmul is ~1/4 rate.

  Membrane chain is split into two independent batch-row halves
  [128, 512] so the serial acc->mem->acc dependency pipelines on DVE
  (chain ~46us < 77us throughput).

Per step, on a [128, 1024] step-tile (2 batch x 512 d per partition):
  DVE  STT x2: acc_h = (mem_h * tau) + x_h          (per half, f32)
  DVE  STT x2: mem_h' = (acc_h <= vth) * acc_h      (per half, f32)
  ACT  Sign:   spk_u8 = u8(sign(acc - vth))         (uint8 out, exact)
Ring schedule per pass: all 16x1MiB loads issue first on the sync
HWDGE ring; per-group 512KiB uint8 spike stores ride the scalar ring
concurrently.  Full-x SBUF residency (bufs=NG).
"""

import os
import sys

sys.path.insert(0, "/opt/trn_rl_repo")

import numpy as np

from concourse import bacc, mybir, tile
from concourse.bass_utils import run_bass_kernel_spmd

TAU = 0.2
VTH = 0.5

B, T, D = 16, 32, 65536
N_CORES = 8
B_SH = B // N_CORES          # 2 batch rows per core
P = 128                      # SBUF partitions
FB = D // P                  # 512 d-elems per partition per batch row
F = B_SH * FB                # 1024 free elems per step-tile

GS = int(os.environ.get("LIF_GS", "4"))   # timesteps per DMA group
NG = T // GS                 # groups per pass
JG = GS * F                  # per-group free elems (4096)

SPK_DT = os.environ.get("LIF_SPKDT", "u8")   # u8 | f8 | bf16
SPLIT0 = int(os.environ.get("LIF_SPLIT0", "0"))  # per-step DMAs for group 0
ST_RING = os.environ.get("LIF_STRING", "scalar")  # sync | scalar | pool
ACC_FULL = int(os.environ.get("LIF_ACCFULL", "0"))  # acc as one [P,F] op
MEM_FULL = int(os.environ.get("LIF_MEMFULL", "0"))  # mem as one [P,F] op

_progs = {}


def _spk_dt():
    return {"u8": mybir.dt.uint8, "f8": mybir.dt.float8e4,
            "bf16": mybir.dt.bfloat16}[SPK_DT]


def _build_program(hw_loop=None, mode="full"):
    f32 = mybir.dt.float32
    nc = bacc.Bacc(
        "TRN2",
        target_bir_lowering=False,
        debug=False,
        enable_asserts=False,
        num_devices=N_CORES,
    )
    x = nc.dram_tensor("x", [B_SH, T, D], f32, kind="ExternalInput").ap()
    xr = x.rearrange("b (g tl) (p f) -> g p tl b f", tl=GS, p=P)
    out = nc.dram_tensor("out", [NG, P, JG], _spk_dt(),
                         kind="ExternalOutput").ap()

    with tile.TileContext(nc) as tc:
        with (
            # full-x residency: one buffer per group, reused across passes
            tc.tile_pool(name="xt", bufs=NG) as xp,
            tc.tile_pool(name="acc", bufs=3) as ap_,
            tc.tile_pool(name="spk", bufs=3) as kp,
            tc.tile_pool(name="m0", bufs=2) as mp0,
            tc.tile_pool(name="m1", bufs=2) as mp1,
            tc.tile_pool(name="const", bufs=1) as cp,
        ):
            nvth = cp.tile([P, 1], f32)
            nc.gpsimd.memset(nvth[:], -VTH)

            def body():
                one_pass(nc, xr, out, xp, ap_, kp, (mp0, mp1),
                         nvth, mode)

            if hw_loop is None:
                body()
            else:
                # benchmarking only: repeat the full pass in a HW loop so
                # per-pass device time can be fit from wall-clock deltas
                with tc.For_i(0, hw_loop, 1):
                    body()
    nc.compile()
    return nc


def one_pass(nc, xr, out, xp, ap_, kp, mps, nvth, mode):
    f32 = mybir.dt.float32
    mult = mybir.AluOpType.mult
    add = mybir.AluOpType.add
    is_le = mybir.AluOpType.is_le
    Sign = mybir.ActivationFunctionType.Sign

    # Phase 1: issue every load on the sync ring (back-to-back, no WAR).
    # Group 0 loads are split per-step so the first acc can start after
    # a 256 KiB DMA instead of a 1 MiB one (ramp cut).
    xts = []
    for g in range(NG):
        xt = xp.tile([P, JG], f32)
        if mode == "compute":
            nc.gpsimd.memset(xt[:], 0.125)
        else:
            xt_v = xt[:].rearrange("p (tl b f) -> p tl b f", tl=GS, b=B_SH)
            if g == 0 and SPLIT0:
                for tl in range(GS):
                    for b in range(B_SH):
                        nc.sync.dma_start(out=xt_v[:, tl, b],
                                          in_=xr[g][:, tl, b])
            else:
                for b in range(B_SH):
                    nc.sync.dma_start(out=xt_v[:, :, b], in_=xr[g][:, :, b])
        xts.append(xt)
    if mode == "load":
        return

    # Phase 2: recurrence; uint8 spike stores drain after loads by FIFO
    mem = [None, None]           # per-half membrane tiles [P, FB]
    for g in range(NG):
        xt = xts[g]
        spk = kp.tile([P, JG], _spk_dt())
        for tl in range(GS):
            t = g * GS + tl
            xs = xt[:, tl * F:(tl + 1) * F]
            if t == 0:
                acc = xs         # mem_{-1} = 0 -> acc = x_0
            else:
                acc = ap_.tile([P, F], f32)
                if ACC_FULL:
                    # one full-tile op; mem is a full tile w/ half writes
                    nc.vector.scalar_tensor_tensor(
                        out=acc[:], in0=mem[0][:], scalar=TAU,
                        in1=xs, op0=mult, op1=add,
                    )
                else:
                    for h in range(B_SH):
                        hs = slice(h * FB, (h + 1) * FB)
                        # acc = (mem * tau) + x_t  (independent per half)
                        nc.vector.scalar_tensor_tensor(
                            out=acc[:, hs],
                            in0=mem[h][:, hs] if (ACC_FULL or MEM_FULL)
                            else mem[h][:], scalar=TAU,
                            in1=xs[:, hs], op0=mult, op1=add,
                        )
            afull = acc if t == 0 else acc[:]
            if t < T - 1:
                if ACC_FULL:
                    m = mps[0].tile([P, F], f32, name="memf")
                    for h in range(B_SH):
                        hs = slice(h * FB, (h + 1) * FB)
                        # mem' = (acc <= vth) * acc   (hard reset)
                        nc.vector.scalar_tensor_tensor(
                            out=m[:, hs], in0=afull[:, hs], scalar=VTH,
                            in1=afull[:, hs], op0=is_le, op1=mult,
                        )
                    mem = [m, m]
                elif MEM_FULL:
                    # one full-tile mem op; acc halves read their slices
                    m = mps[0].tile([P, F], f32, name="memf")
                    nc.vector.scalar_tensor_tensor(
                        out=m[:], in0=afull, scalar=VTH,
                        in1=afull, op0=is_le, op1=mult,
                    )
                    mem = [m, m]
                else:
                    for h in range(B_SH):
                        hs = slice(h * FB, (h + 1) * FB)
                        m = mps[h].tile([P, FB], f32)
                        # mem' = (acc <= vth) * acc   (hard reset)
                        nc.vector.scalar_tensor_tensor(
                            out=m[:], in0=afull[:, hs], scalar=VTH,
                            in1=afull[:, hs], op0=is_le, op1=mult,
                        )
                        mem[h] = m
            # spike = sign(acc-vth) written directly as uint8: the f32->u8
            # conversion saturates -1 to 0, so u8(sgn) is exactly the
            # {0,1} spike (verified incl. threshold-boundary values)
            nc.scalar.activation(spk[:, tl * F:(tl + 1) * F], afull,
                                 Sign, bias=nvth[:])
        if mode == "full":
            st_eng = {"sync": nc.sync, "scalar": nc.scalar,
                      "pool": nc.gpsimd}[ST_RING]
            st_eng.dma_start(out=out[g], in_=spk[:])


def _get_program(hw_loop=None, mode="full"):
    key = (hw_loop, mode)
    if key not in _progs:
        _progs[key] = _build_program(hw_loop, mode)
    return _progs[key]


# ---- host-side shard/gather ------------------------------------------

def _shard_input(xc):
    return np.ascontiguousarray(xc)


def _gather_output(oc):
    """[NG, P, JG] spike-dtype -> [B_SH, T, D] f32 spikes (exact)."""
    oc = np.asarray(oc)
    if oc.dtype == np.uint8 and SPK_DT == "f8":
        oc = (oc != 0)
    elif SPK_DT == "f8":
        oc = (np.asarray(oc).view(np.uint8) != 0)
    sp = oc.reshape(NG, P, GS, B_SH, FB).transpose(3, 0, 2, 1, 4)
    return np.ascontiguousarray(
        sp.reshape(B_SH, T, D).astype(np.float32)
    )


def device_input(x):
    """Full [B, T, D] -> axis-0 shard-concatenated device input array."""
    return np.ascontiguousarray(np.asarray(x, dtype=np.float32))


def device_output(o):
    """Axis-0 shard-concatenated device output -> full [B, T, D] f32."""
    rows = o.shape[0] // N_CORES
    return np.concatenate(
        [
            _gather_output(o[i * rows:(i + 1) * rows])
            for i in range(N_CORES)
        ],
        axis=0,
    )


def _shard(x):
    return [
        {"x": _shard_input(x[i * B_SH:(i + 1) * B_SH])}
        for i in range(N_CORES)
    ]


def kernel(x):
    x = np.asarray(x, dtype=np.float32)
    assert x.shape == (B, T, D), x.shape
    nc = _get_program()
    res = run_bass_kernel_spmd(nc, _shard(x), list(range(N_CORES)))
    return np.concatenate(
        [_gather_output(res.results[i]["out"]) for i in range(N_CORES)],
        axis=0,
    )
